# revision 22
# baseline (speedup 1.0000x reference)
"""AlignedTripletLoss Trainium2 kernel (8 NeuronCores, symmetric block-pair
decomposition).

Math (matches reference.py):
  x_hat = x / (||x||_2 + 1e-12) per (image, part) row               [1024*8, 128]
  dist2[(a,i),(b,j)] = 2 - 2 * <x_hat_(a,i), x_hat_(b,j)>  (rows are unit norm,
      so the sq-norm terms are 1 up to ~1e-6; a +4e-4 bias keeps sqrt's argument
      positive on the diagonal despite float32r matmul rounding)
  t = tanh(0.5 * sqrt(dist2))
  dtw[a,b] = monotone (right/down) shortest path over the 8x8 grid t[i][j]
  ap = max over positives, an = min over negatives, loss = mean(relu(ap-an+0.3))

Sharding: dtw is symmetric (the DTW of the transposed cell grid is the mirrored
path set), so only unordered image-block pairs need computing. Core k computes
blocks (k, (k+d) mod 8) for d = 0..4 -- a uniform circulant cover of all 36
unordered pairs (d=4 pairs are computed twice; min/max mining makes duplicates
harmless). That is 640 columns per core instead of 1024 (37.5% less DTW work).
Each core row-mines its own anchors over its 640 columns AND, after a PE
transpose of each off-diagonal [128,128] dtw block, column-mines the partner
block's anchors. Per-core output is 10 partial min/max vectors; the host glue
combines partials per anchor (placement by core id) and takes the mean -- the
analogue of the sharding hint's final all-reduce.

The DTW row recurrence val[j] = min(val[j-1], up[j]) + t[i][j] is exactly DVE
tensor_tensor_scan(op0=min, op1=add) along the free axis, with a dummy element
between consecutive (a,b) pairs to reset the running state (data0 dummy -BIG
with data1 dummy +BIG resets to 0 on row 0; prior-row outputs regenerate the
reset for rows 1..7 automatically).

Perf notes:
 - pairwise dots run as float32r matmuls (1 cyc/col at free dim >= 256);
   inputs are rounded to f32r by the producing copies as walrus requires.
 - normalization scale is folded into the transpose: PE matmul against a
   gpsimd-built diag(1/norm) transposes and scales in one pass.
 - sqrt reads PSUM with a scattered (b,j) AP (free) and writes the scan layout
   in 32B runs (measured full rate); tanh runs in place on the same layout.
 - row buffers live in a persistent arena whose scan-reset dummies are memset
   exactly once.
"""

import numpy as np

N, M, D = 1024, 8, 128
MARGIN = 0.3
EPS = 1e-12
NCORES = 8
A = N // NCORES          # anchors per core (one image block)
NDIAG = 5                # circulant depth: blocks k..k+4
NCOL = NDIAG * A         # 640 columns per core
CBS = [256, 256, 128]    # column batches (blocks d=0,1 | d=2,3 | d=4)
CBMAX = 256
G = M + 1                # scan group: 1 dummy + 8 j-steps
BIG = 1e9
SQ_BIAS = 2.0 + 4e-4

_CACHE = {}


def _build_nc():
    import concourse.bacc as bacc
    import concourse.mybir as mybir
    import concourse.tile as tile
    from concourse.tile import add_dep_helper
    from concourse.masks import make_identity

    fp32 = mybir.dt.float32
    f32r = mybir.dt.float32r
    AF = mybir.ActivationFunctionType
    OP = mybir.AluOpType
    AX = mybir.AxisListType

    nc = bacc.Bacc("TRN2", target_bir_lowering=False, debug=False,
                   num_devices=NCORES)

    xa_in = nc.dram_tensor("xa", [A * M, D], fp32, kind="ExternalInput")
    xr_in = nc.dram_tensor("xr5", [NCOL * M, D], fp32, kind="ExternalInput")
    mop_in = nc.dram_tensor("m_own_pos", [A, NCOL], fp32, kind="ExternalInput")
    mon_in = nc.dram_tensor("m_own_neg", [A, NCOL], fp32, kind="ExternalInput")
    mtp_in = nc.dram_tensor("m_t_pos", [A, (NDIAG - 1) * A], fp32,
                            kind="ExternalInput")
    mtn_in = nc.dram_tensor("m_t_neg", [A, (NDIAG - 1) * A], fp32,
                            kind="ExternalInput")
    out_t = nc.dram_tensor("partials", [A, 10], fp32, kind="ExternalOutput")

    S = (NCOL * M) // 128   # 40 row-tiles of xr5
    SA = (A * M) // 128     # 8 row-tiles of xa
    NB = len(CBS)

    with tile.TileContext(nc) as tc:
        with tc.tile_pool(name="persist", bufs=1) as persist:
            xrT = persist.tile([128, NCOL, M], f32r)  # x_hat^T [d][b][j], b-major
            xTa = persist.tile([128, M, A], f32r)     # -2*x_hat_anchor^T [d][i][a]
            mop = persist.tile([128, NCOL], fp32)
            mon = persist.tile([128, NCOL], fp32)
            mtp = persist.tile([128, (NDIAG - 1) * A], fp32)
            mtn = persist.tile([128, (NDIAG - 1) * A], fp32)
            up0 = persist.tile([128, CBMAX * G], fp32)
            biasT = persist.tile([128, 1], fp32)
            dtwc = persist.tile([128, NCOL], fp32)    # compact dtw row block
            apacc = persist.tile([128, NB], fp32)
            anacc = persist.tile([128, NB], fp32)
            pout = persist.tile([128, 10], fp32)
            ident = persist.tile([128, 128], fp32)
            RSLOT = 9
            arena = persist.tile([128, RSLOT, CBMAX, G], fp32)

            nc.sync.dma_start(mop[:], mop_in[:])
            nc.sync.dma_start(mon[:], mon_in[:])
            nc.sync.dma_start(mtp[:], mtp_in[:])
            nc.sync.dma_start(mtn[:], mtn_in[:])
            nc.gpsimd.memset(biasT[:], SQ_BIAS)
            up0v = up0.rearrange("p (c g) -> p c g", g=G)
            nc.gpsimd.memset(up0v[:, :, 0:1], -BIG)
            nc.gpsimd.memset(up0v[:, :, 1:G], BIG)
            nc.gpsimd.memset(arena[:, :, :, 0:1], BIG)
            make_identity(nc, ident[:])

            # ---------- setup: normalize + transpose (scale fused via diag) ----
            # Pools stay open (co-resident with main loop) so main-loop tiles
            # get distinct addresses and can overlap setup execution.
            with (
                tc.tile_pool(name="setup", bufs=1) as setup,
                tc.tile_pool(name="chunk", bufs=2) as chunk,
                tc.tile_pool(name="dgp", bufs=3) as dgp,
                tc.tile_pool(name="psump", bufs=2, space="PSUM") as psump,
                tc.tile_pool(name="valsp", bufs=2) as valsp,
                tc.tile_pool(name="mtmp", bufs=3) as mtmp,
            ):
                def norm_rn(src_dram, n_tiles, neg2, tagp, src_ap=None):
                    """Rows p-outer (row r = p*n_tiles + s); rn = 1/(||row||+eps)."""
                    xr = setup.tile([128, n_tiles, D], fp32, tag=f"xr{tagp}")
                    if src_ap is None:
                        nc.sync.dma_start(
                            xr[:], src_dram.rearrange("(p s) d -> p s d", p=128))
                    else:
                        xrv = xr.rearrange("p (blk s) d -> p blk s d", s=M)
                        nblk = n_tiles // M
                        for b0 in range(0, nblk, 2):
                            b1 = min(b0 + 2, nblk)
                            nc.sync.dma_start(
                                xrv[:, b0:b1, :, :], src_ap[:, b0:b1, :, :])
                    n2 = setup.tile([128, n_tiles], fp32, tag=f"n2{tagp}")
                    nrm = setup.tile([128, n_tiles], fp32, tag=f"nr{tagp}")
                    rn = setup.tile([128, n_tiles], fp32, tag=f"rn{tagp}")
                    for g in range(0, n_tiles, 16):
                        CH = min(16, n_tiles - g)
                        x2 = chunk.tile([128, 16, D], fp32, tag="x2c")
                        nc.scalar.activation(
                            x2[:, :CH, :], xr[:, g:g + CH, :], AF.Square)
                        nc.vector.tensor_reduce(
                            n2[:, g:g + CH], x2[:, :CH, :], axis=AX.X, op=OP.add)
                        nc.scalar.activation(
                            nrm[:, g:g + CH], n2[:, g:g + CH], AF.Sqrt)
                        nc.vector.tensor_scalar_add(
                            nrm[:, g:g + CH], nrm[:, g:g + CH], EPS)
                        nc.vector.reciprocal(
                            rn[:, g:g + CH], nrm[:, g:g + CH])
                        if neg2:
                            nc.vector.tensor_scalar_mul(
                                rn[:, g:g + CH], rn[:, g:g + CH], -2.0)
                    return xr, rn

                def diag4(rn, s0):
                    dgc = dgp.tile([128, 4, 128], fp32, tag="dgc")
                    for jj in range(4):
                        nc.gpsimd.affine_select(
                            out=dgc[:, jj, :],
                            in_=rn[:, s0 + jj:s0 + jj + 1].to_broadcast((128, 128)),
                            compare_op=OP.is_equal, fill=0.0, base=0,
                            pattern=[[-1, 128]], channel_multiplier=1)
                    return dgc

                xra, rna = norm_rn(xa_in, SA, neg2=True, tagp="a")
                for half in range(2):
                    dgc = diag4(rna, 4 * half)
                    pts = psump.tile([128, M, CBMAX], fp32, tag="pp",
                                     name="ptslot")
                    pt = pts.rearrange("p j b -> p (j b)").rearrange(
                        "p (g v) -> p g v", v=128)[:, 0:4, :]
                    for jj in range(4):
                        s = 4 * half + jj
                        nc.tensor.matmul(
                            pt[:, jj, :], lhsT=xra[:, s, :],
                            rhs=dgc[:, jj, :], start=True, stop=True)
                    # tile s holds rows r = p*8+s -> (a=p, i=s)
                    dst = xTa[:, 4 * half:4 * half + 4, :]
                    nc.scalar.activation(dst, pt[:], AF.Copy)

                # xr5 laid out per block: tile t = blk*8 + s holds rows
                # blk*1024 + p*8 + s -> (col = blk*128 + p, j = s), so early
                # blocks complete first and batch-0 matmuls start sooner.
                xr, rn = norm_rn(
                    xr_in, S, neg2=False, tagp="x",
                    src_ap=xr_in.rearrange(
                        "(blk p s) d -> p blk s d", p=128, s=M))
                for blk in range(NDIAG):
                    for half in range(2):
                        dgc = diag4(rn, 8 * blk + 4 * half)
                        pts = psump.tile([128, M, CBMAX], fp32, tag="pp",
                                         name="ptslot")
                        pt = pts.rearrange("p j b -> p (j b)").rearrange(
                            "p (g v) -> p g v", v=128)[:, 0:4, :]
                        for jj in range(4):
                            s = 8 * blk + 4 * half + jj
                            nc.tensor.matmul(
                                pt[:, jj, :], lhsT=xr[:, s, :],
                                rhs=dgc[:, jj, :], start=True, stop=True)
                        dst = xrT[:, blk * A:(blk + 1) * A,
                                  4 * half:4 * half + 4]
                        nc.scalar.activation(
                            dst, pt.rearrange("d j b -> d b j"), AF.Copy)


                # ---------- main loop (high priority: scheduler interleaves
                # remaining setup into engine gaps instead of front-running) --
                mpsum = psump
                prev_tanh_last = None
                slot = 0
                col0 = 0
                tblocks = [[1], [2, 3], [4]]
                for n in range(NB):
                    CB = CBS[n]
                    sd = []
                    sqrt_insts = []
                    for i in range(M):
                        pp = mpsum.tile([128, M, CBMAX], fp32, tag="pp")
                        for j in range(M):
                            nc.tensor.matmul(
                                pp[:, j, :CB], lhsT=xTa[:, i, :],
                                rhs=xrT[:, col0:col0 + CB, j],
                                start=True, stop=True)
                        buf = arena[:, slot, :CB, :]
                        slot = (slot + 1) % RSLOT
                        # read PSUM scattered in (b, j) order; write 32B runs
                        inst = nc.scalar.activation(
                            buf[:, :, 1:G],
                            pp[:, :, :CB].rearrange("p j b -> p b j"),
                            AF.Sqrt, bias=biasT[:, 0:1])
                        if prev_tanh_last is not None:
                            add_dep_helper(inst.ins, prev_tanh_last.ins,
                                           sync=False,
                                           reason="ACT table batch order")
                        sqrt_insts.append(inst)
                        sd.append(buf)
                    for i in range(M):
                        v = sd[i][:, :, 1:G]
                        t_inst = nc.scalar.activation(v, v, AF.Tanh, scale=0.5)
                        add_dep_helper(t_inst.ins, sqrt_insts[-1].ins,
                                       sync=False,
                                       reason="ACT table batch order")
                        prev_tanh_last = t_inst
                    prev_vals = None
                    for i in range(M):
                        vt = valsp.tile([128, CBMAX * G], fp32, tag="vals")
                        d0 = up0[:, :CB * G] if i == 0 else prev_vals[:, :CB * G]
                        nc.vector.tensor_tensor_scan(
                            vt[:, :CB * G], d0,
                            sd[i].rearrange("p c g -> p (c g)"),
                            0.0, OP.min, OP.add)
                        prev_vals = vt
                    dtw = prev_vals.rearrange(
                        "p (c g) -> p c g", g=G)[:, :CB, M:M + 1]
                    dtw = dtw.rearrange("p c o -> p (c o)")
                    # compact copy (feeds block transposes + mining)
                    nc.vector.tensor_copy(dtwc[:, col0:col0 + CB], dtw)
                    tp = mtmp.tile([128, CBMAX], fp32, tag="tp")
                    nc.vector.tensor_tensor(
                        tp[:, :CB], dtwc[:, col0:col0 + CB],
                        mop[:, col0:col0 + CB], OP.add)
                    nc.vector.tensor_reduce(
                        apacc[:, n:n + 1], tp[:, :CB], axis=AX.X, op=OP.max)
                    tn = mtmp.tile([128, CBMAX], fp32, tag="tn")
                    nc.vector.tensor_tensor(
                        tn[:, :CB], dtwc[:, col0:col0 + CB],
                        mon[:, col0:col0 + CB], OP.add)
                    nc.vector.tensor_reduce(
                        anacc[:, n:n + 1], tn[:, :CB], axis=AX.X, op=OP.min)
                    col0 += CB

                # own-anchor partials -> pout cols 0 (an), 1 (ap)
                nc.vector.tensor_reduce(
                    pout[:, 0:1], anacc[:], axis=AX.X, op=OP.min)
                nc.vector.tensor_reduce(
                    pout[:, 1:2], apacc[:], axis=AX.X, op=OP.max)

                # transposed blocks d=1..4: partner anchors over our columns
                for d in range(1, NDIAG):
                    ptps = mpsum.tile([128, M, CBMAX], fp32, tag="pp",
                                      name="ptpslot")
                    ptp = ptps.rearrange("p j b -> p (j b)")[:, :128]
                    nc.tensor.transpose(
                        ptp[:], dtwc[:, d * A:(d + 1) * A], ident[:])
                    tb = mtmp.tile([128, 128], fp32, tag="tb")
                    nc.vector.tensor_copy(tb[:], ptp[:])
                    tpp = mtmp.tile([128, 128], fp32, tag="tpp")
                    nc.vector.tensor_tensor(
                        tpp[:], tb[:], mtp[:, (d - 1) * A:d * A], OP.add)
                    nc.vector.tensor_reduce(
                        pout[:, 2 * d + 1:2 * d + 2], tpp[:],
                        axis=AX.X, op=OP.max)
                    nc.vector.tensor_tensor(
                        tpp[:], tb[:], mtn[:, (d - 1) * A:d * A], OP.add)
                    nc.vector.tensor_reduce(
                        pout[:, 2 * d:2 * d + 1], tpp[:],
                        axis=AX.X, op=OP.min)

                nc.sync.dma_start(out_t[:], pout[:])

    nc.compile()
    return nc


def _get_nc():
    if "nc" not in _CACHE:
        _CACHE["nc"] = _build_nc()
    return _CACHE["nc"]


def kernel(inputs, labels, _trace=False, _trace_cores=None):
    from concourse.bass_utils import run_bass_kernel_spmd

    x = np.ascontiguousarray(np.asarray(inputs, dtype=np.float32)).reshape(N * M, D)
    lab = np.asarray(labels)

    nc = _get_nc()
    in_maps = []
    for c in range(NCORES):
        blocks = [(c + d) % NCORES for d in range(NDIAG)]
        col_img = np.concatenate([np.arange(b * A, (b + 1) * A) for b in blocks])
        row_img = np.arange(c * A, (c + 1) * A)
        xr5 = np.ascontiguousarray(
            x.reshape(N, M, D)[col_img].reshape(NCOL * M, D))
        xa = np.ascontiguousarray(x[c * A * M:(c + 1) * A * M])
        eq_own = lab[row_img][:, None] == lab[col_img][None, :]
        m_own_pos = np.where(eq_own, np.float32(0.0), np.float32(-1e30))
        m_own_neg = np.where(eq_own, np.float32(1e30), np.float32(0.0))
        # transposed blocks: anchors = block (c+d)%8, cols = block c images
        mtp_l, mtn_l = [], []
        for d in range(1, NDIAG):
            arow = lab[np.arange(blocks[d] * A, (blocks[d] + 1) * A)]
            eq_t = arow[:, None] == lab[row_img][None, :]
            mtp_l.append(np.where(eq_t, np.float32(0.0), np.float32(-1e30)))
            mtn_l.append(np.where(eq_t, np.float32(1e30), np.float32(0.0)))
        in_maps.append({
            "xa": xa,
            "xr5": xr5,
            "m_own_pos": np.ascontiguousarray(m_own_pos.astype(np.float32)),
            "m_own_neg": np.ascontiguousarray(m_own_neg.astype(np.float32)),
            "m_t_pos": np.ascontiguousarray(
                np.concatenate(mtp_l, axis=1).astype(np.float32)),
            "m_t_neg": np.ascontiguousarray(
                np.concatenate(mtn_l, axis=1).astype(np.float32)),
        })
    res = run_bass_kernel_spmd(
        nc, in_maps, core_ids=list(range(NCORES)), trace=_trace,
        trace_cores=_trace_cores)
    if _trace:
        _CACHE["last_results"] = res

    # host glue: combine per-core min/max partials per anchor block
    an_all = np.full((NCORES, A), np.inf, dtype=np.float32)
    ap_all = np.full((NCORES, A), -np.inf, dtype=np.float32)
    for c in range(NCORES):
        p = res.results[c]["partials"]  # [A, 10]
        for d in range(NDIAG):
            blk = (c + d) % NCORES
            an_all[blk] = np.minimum(an_all[blk], p[:, 2 * d])
            ap_all[blk] = np.maximum(ap_all[blk], p[:, 2 * d + 1])
    loss_vec = np.maximum(
        ap_all.reshape(-1) - an_all.reshape(-1) + np.float32(MARGIN),
        np.float32(0.0))
    return np.asarray(loss_vec.mean(), dtype=np.float32)


# revision 23
# speedup vs baseline: 1.1457x; 1.1457x over previous
"""AlignedTripletLoss Trainium2 kernel (8 NeuronCores, symmetric block-pair
decomposition).

Math (matches reference.py):
  x_hat = x / (||x||_2 + 1e-12) per (image, part) row               [1024*8, 128]
  dist2[(a,i),(b,j)] = 2 - 2 * <x_hat_(a,i), x_hat_(b,j)>  (rows are unit norm,
      so the sq-norm terms are 1 up to ~1e-6; a +4e-4 bias keeps sqrt's argument
      positive on the diagonal despite float32r matmul rounding)
  t = tanh(0.5 * sqrt(dist2))
  dtw[a,b] = monotone (right/down) shortest path over the 8x8 grid t[i][j]
  ap = max over positives, an = min over negatives, loss = mean(relu(ap-an+0.3))

Sharding: dtw is symmetric (the DTW of the transposed cell grid is the mirrored
path set), so only unordered image-block pairs need computing. Core k computes
blocks (k, (k+d) mod 8) for d = 0..4 -- a uniform circulant cover of all 36
unordered pairs (d=4 pairs are computed twice; min/max mining makes duplicates
harmless). That is 640 columns per core instead of 1024 (37.5% less DTW work).
Each core row-mines its own anchors over its 640 columns AND, after a PE
transpose of each off-diagonal [128,128] dtw block, column-mines the partner
block's anchors. Per-core output is 10 partial min/max vectors; the host glue
combines partials per anchor (placement by core id) and takes the mean -- the
analogue of the sharding hint's final all-reduce.

The DTW row recurrence val[j] = min(val[j-1], up[j]) + t[i][j] is exactly DVE
tensor_tensor_scan(op0=min, op1=add) along the free axis, with a dummy element
between consecutive (a,b) pairs to reset the running state (data0 dummy -BIG
with data1 dummy +BIG resets to 0 on row 0; prior-row outputs regenerate the
reset for rows 1..7 automatically).

Perf notes:
 - pairwise dots run as float32r matmuls (1 cyc/col at free dim >= 256);
   inputs are rounded to f32r by the producing copies as walrus requires.
 - normalization scale is folded into the transpose: PE matmul against a
   gpsimd-built diag(1/norm) transposes and scales in one pass.
 - sqrt reads PSUM with a scattered (b,j) AP (free) and writes the scan layout
   in 32B runs (measured full rate); tanh runs in place on the same layout.
 - row buffers live in a persistent arena whose scan-reset dummies are memset
   exactly once.
"""

import numpy as np

N, M, D = 1024, 8, 128
MARGIN = 0.3
EPS = 1e-12
NCORES = 8
A = N // NCORES          # anchors per core (one image block)
NDIAG = 5                # circulant depth: blocks k..k+4
NCOL = NDIAG * A         # 640 columns per core
CBS = [256, 256, 128]    # column batches (blocks d=0,1 | d=2,3 | d=4)
CBMAX = 256
G = M + 1                # scan group: 1 dummy + 8 j-steps
BIG = 1e9
SQ_BIAS = 2.0 + 4e-4

_CACHE = {}


def _build_nc():
    import concourse.bacc as bacc
    import concourse.mybir as mybir
    import concourse.tile as tile
    from concourse.tile import add_dep_helper
    from concourse.masks import make_identity

    fp32 = mybir.dt.float32
    f32r = mybir.dt.float32r
    AF = mybir.ActivationFunctionType
    OP = mybir.AluOpType
    AX = mybir.AxisListType

    nc = bacc.Bacc("TRN2", target_bir_lowering=False, debug=False,
                   num_devices=NCORES)

    xa_in = nc.dram_tensor("xa", [A * M, D], fp32, kind="ExternalInput")
    xr_in = nc.dram_tensor("xr5", [NCOL * M, D], fp32, kind="ExternalInput")
    mop_in = nc.dram_tensor("m_own_pos", [A, NCOL], fp32, kind="ExternalInput")
    mon_in = nc.dram_tensor("m_own_neg", [A, NCOL], fp32, kind="ExternalInput")
    mtp_in = nc.dram_tensor("m_t_pos", [A, (NDIAG - 1) * A], fp32,
                            kind="ExternalInput")
    mtn_in = nc.dram_tensor("m_t_neg", [A, (NDIAG - 1) * A], fp32,
                            kind="ExternalInput")
    out_t = nc.dram_tensor("partials", [A, 10], fp32, kind="ExternalOutput")

    S = (NCOL * M) // 128   # 40 row-tiles of xr5
    SA = (A * M) // 128     # 8 row-tiles of xa
    NB = len(CBS)

    with tile.TileContext(nc) as tc:
        with tc.tile_pool(name="persist", bufs=1) as persist:
            xrT = persist.tile([128, NCOL, M], f32r)  # x_hat^T [d][b][j], b-major
            xTa = persist.tile([128, M, A], f32r)     # -2*x_hat_anchor^T [d][i][a]
            mop = persist.tile([128, NCOL], fp32)
            mon = persist.tile([128, NCOL], fp32)
            mtp = persist.tile([128, (NDIAG - 1) * A], fp32)
            mtn = persist.tile([128, (NDIAG - 1) * A], fp32)
            up0 = persist.tile([128, CBMAX * G], fp32)
            biasT = persist.tile([128, 1], fp32)
            dtwc = persist.tile([128, NCOL], fp32)    # compact dtw row block
            apacc = persist.tile([128, NB], fp32)
            anacc = persist.tile([128, NB], fp32)
            pout = persist.tile([128, 10], fp32)
            ident = persist.tile([128, 128], fp32)
            RSLOT = 10
            arena = persist.tile([128, RSLOT, CBMAX, G], fp32)

            nc.sync.dma_start(mop[:], mop_in[:])
            nc.sync.dma_start(mon[:], mon_in[:])
            nc.sync.dma_start(mtp[:], mtp_in[:])
            nc.sync.dma_start(mtn[:], mtn_in[:])
            nc.gpsimd.memset(biasT[:], SQ_BIAS)
            up0v = up0.rearrange("p (c g) -> p c g", g=G)
            nc.gpsimd.memset(up0v[:, :, 0:1], -BIG)
            nc.gpsimd.memset(up0v[:, :, 1:G], BIG)
            nc.gpsimd.memset(arena[:, :, :, 0:1], BIG)
            make_identity(nc, ident[:])

            # ---------- setup: normalize + transpose (scale fused via diag) ----
            with (
                tc.tile_pool(name="setup", bufs=1) as setup,
                tc.tile_pool(name="chunk", bufs=2) as chunk,
                tc.tile_pool(name="dgp", bufs=3) as dgp,
                tc.tile_pool(name="tpsum", bufs=2, space="PSUM") as tpsum,
            ):
                def norm_rn(src_dram, n_tiles, neg2, tagp, src_ap=None):
                    """Rows p-outer (row r = p*n_tiles + s); rn = 1/(||row||+eps)."""
                    xr = setup.tile([128, n_tiles, D], fp32, tag=f"xr{tagp}")
                    if src_ap is None:
                        src_ap = src_dram.rearrange("(p s) d -> p s d", p=128)
                        nc.sync.dma_start(xr[:], src_ap)
                    else:
                        nc.sync.dma_start(
                            xr.rearrange("p (blk s) d -> p blk s d", s=M),
                            src_ap)
                    n2 = setup.tile([128, n_tiles], fp32, tag=f"n2{tagp}")
                    nrm = setup.tile([128, n_tiles], fp32, tag=f"nr{tagp}")
                    rn = setup.tile([128, n_tiles], fp32, tag=f"rn{tagp}")
                    for g in range(0, n_tiles, 16):
                        CH = min(16, n_tiles - g)
                        x2 = chunk.tile([128, 16, D], fp32, tag="x2c")
                        nc.scalar.activation(
                            x2[:, :CH, :], xr[:, g:g + CH, :], AF.Square)
                        nc.vector.tensor_reduce(
                            n2[:, g:g + CH], x2[:, :CH, :], axis=AX.X, op=OP.add)
                        nc.scalar.activation(
                            nrm[:, g:g + CH], n2[:, g:g + CH], AF.Sqrt)
                        nc.vector.tensor_scalar_add(
                            nrm[:, g:g + CH], nrm[:, g:g + CH], EPS)
                        nc.vector.reciprocal(
                            rn[:, g:g + CH], nrm[:, g:g + CH])
                        if neg2:
                            nc.vector.tensor_scalar_mul(
                                rn[:, g:g + CH], rn[:, g:g + CH], -2.0)
                    return xr, rn

                def diag4(rn, s0):
                    dgc = dgp.tile([128, 4, 128], fp32, tag="dgc")
                    for jj in range(4):
                        nc.gpsimd.affine_select(
                            out=dgc[:, jj, :],
                            in_=rn[:, s0 + jj:s0 + jj + 1].to_broadcast((128, 128)),
                            compare_op=OP.is_equal, fill=0.0, base=0,
                            pattern=[[-1, 128]], channel_multiplier=1)
                    return dgc

                xra, rna = norm_rn(xa_in, SA, neg2=True, tagp="a")
                for half in range(2):
                    dgc = diag4(rna, 4 * half)
                    pt = tpsum.tile([128, 4, 128], fp32, tag="tp")
                    for jj in range(4):
                        s = 4 * half + jj
                        nc.tensor.matmul(
                            pt[:, jj, :], lhsT=xra[:, s, :],
                            rhs=dgc[:, jj, :], start=True, stop=True)
                    # tile s holds rows r = p*8+s -> (a=p, i=s)
                    dst = xTa[:, 4 * half:4 * half + 4, :]
                    nc.scalar.activation(dst, pt[:], AF.Copy)

                # xr5 laid out per block: tile t = blk*8 + s holds rows
                # blk*1024 + p*8 + s -> (col = blk*128 + p, j = s), so early
                # blocks complete first and batch-0 matmuls start sooner.
                xr, rn = norm_rn(
                    xr_in, S, neg2=False, tagp="x",
                    src_ap=xr_in.rearrange(
                        "(blk p s) d -> p blk s d", p=128, s=M))
                for blk in range(NDIAG):
                    for half in range(2):
                        dgc = diag4(rn, 8 * blk + 4 * half)
                        pt = tpsum.tile([128, 4, 128], fp32, tag="tp")
                        for jj in range(4):
                            s = 8 * blk + 4 * half + jj
                            nc.tensor.matmul(
                                pt[:, jj, :], lhsT=xr[:, s, :],
                                rhs=dgc[:, jj, :], start=True, stop=True)
                        dst = xrT[:, blk * A:(blk + 1) * A,
                                  4 * half:4 * half + 4]
                        nc.scalar.activation(
                            dst, pt.rearrange("d j b -> d b j"), AF.Copy)


            # ---------- main loop ----------
            with (
                tc.tile_pool(name="valsp", bufs=3) as valsp,
                tc.tile_pool(name="mtmp", bufs=6) as mtmp,
                tc.tile_pool(name="mpsum", bufs=2, space="PSUM") as mpsum,
            ):
                prev_tanh_last = None
                slot = 0
                col0 = 0
                tblocks = [[1], [2, 3], [4]]
                for n in range(NB):
                    CB = CBS[n]
                    sd = []
                    sqrt_insts = []
                    for i in range(M):
                        pp = mpsum.tile([128, M, CBMAX], fp32, tag="pp")
                        for j in range(M):
                            nc.tensor.matmul(
                                pp[:, j, :CB], lhsT=xTa[:, i, :],
                                rhs=xrT[:, col0:col0 + CB, j],
                                start=True, stop=True)
                        buf = arena[:, slot, :CB, :]
                        slot = (slot + 1) % RSLOT
                        # read PSUM scattered in (b, j) order; write 32B runs
                        inst = nc.scalar.activation(
                            buf[:, :, 1:G],
                            pp[:, :, :CB].rearrange("p j b -> p b j"),
                            AF.Sqrt, bias=biasT[:, 0:1])
                        if prev_tanh_last is not None:
                            add_dep_helper(inst.ins, prev_tanh_last.ins,
                                           sync=False,
                                           reason="ACT table batch order")
                        sqrt_insts.append(inst)
                        sd.append(buf)
                    for i in range(M):
                        v = sd[i][:, :, 1:G]
                        t_inst = nc.scalar.activation(v, v, AF.Tanh, scale=0.5)
                        add_dep_helper(t_inst.ins, sqrt_insts[-1].ins,
                                       sync=False,
                                       reason="ACT table batch order")
                        prev_tanh_last = t_inst
                    prev_vals = None
                    for i in range(M):
                        vt = valsp.tile([128, CBMAX * G], fp32, tag="vals")
                        d0 = up0[:, :CB * G] if i == 0 else prev_vals[:, :CB * G]
                        nc.vector.tensor_tensor_scan(
                            vt[:, :CB * G], d0,
                            sd[i].rearrange("p c g -> p (c g)"),
                            0.0, OP.min, OP.add)
                        prev_vals = vt
                    dtw = prev_vals.rearrange(
                        "p (c g) -> p c g", g=G)[:, :CB, M:M + 1]
                    dtw = dtw.rearrange("p c o -> p (c o)")
                    # compact copy (feeds block transposes + mining)
                    nc.vector.tensor_copy(dtwc[:, col0:col0 + CB], dtw)
                    tp = mtmp.tile([128, CBMAX], fp32, tag="tp")
                    nc.vector.tensor_tensor(
                        tp[:, :CB], dtwc[:, col0:col0 + CB],
                        mop[:, col0:col0 + CB], OP.add)
                    nc.vector.tensor_reduce(
                        apacc[:, n:n + 1], tp[:, :CB], axis=AX.X, op=OP.max)
                    tn = mtmp.tile([128, CBMAX], fp32, tag="tn")
                    nc.vector.tensor_tensor(
                        tn[:, :CB], dtwc[:, col0:col0 + CB],
                        mon[:, col0:col0 + CB], OP.add)
                    nc.vector.tensor_reduce(
                        anacc[:, n:n + 1], tn[:, :CB], axis=AX.X, op=OP.min)
                    col0 += CB

                # own-anchor partials -> pout cols 0 (an), 1 (ap)
                nc.vector.tensor_reduce(
                    pout[:, 0:1], anacc[:], axis=AX.X, op=OP.min)
                nc.vector.tensor_reduce(
                    pout[:, 1:2], apacc[:], axis=AX.X, op=OP.max)

                # transposed blocks d=1..4: partner anchors over our columns
                for d in range(1, NDIAG):
                    ptp = mpsum.tile([128, 128], fp32, tag="pp")
                    nc.tensor.transpose(
                        ptp[:], dtwc[:, d * A:(d + 1) * A], ident[:])
                    tb = mtmp.tile([128, 128], fp32, tag="tb")
                    nc.vector.tensor_copy(tb[:], ptp[:])
                    tpp = mtmp.tile([128, 128], fp32, tag="tpp")
                    nc.vector.tensor_tensor(
                        tpp[:], tb[:], mtp[:, (d - 1) * A:d * A], OP.add)
                    nc.vector.tensor_reduce(
                        pout[:, 2 * d + 1:2 * d + 2], tpp[:],
                        axis=AX.X, op=OP.max)
                    nc.vector.tensor_tensor(
                        tpp[:], tb[:], mtn[:, (d - 1) * A:d * A], OP.add)
                    nc.vector.tensor_reduce(
                        pout[:, 2 * d:2 * d + 1], tpp[:],
                        axis=AX.X, op=OP.min)

                nc.sync.dma_start(out_t[:], pout[:])

    nc.compile()
    return nc


def _get_nc():
    if "nc" not in _CACHE:
        _CACHE["nc"] = _build_nc()
    return _CACHE["nc"]


def kernel(inputs, labels, _trace=False, _trace_cores=None):
    from concourse.bass_utils import run_bass_kernel_spmd

    x = np.ascontiguousarray(np.asarray(inputs, dtype=np.float32)).reshape(N * M, D)
    lab = np.asarray(labels)

    nc = _get_nc()
    in_maps = []
    for c in range(NCORES):
        blocks = [(c + d) % NCORES for d in range(NDIAG)]
        col_img = np.concatenate([np.arange(b * A, (b + 1) * A) for b in blocks])
        row_img = np.arange(c * A, (c + 1) * A)
        xr5 = np.ascontiguousarray(
            x.reshape(N, M, D)[col_img].reshape(NCOL * M, D))
        xa = np.ascontiguousarray(x[c * A * M:(c + 1) * A * M])
        eq_own = lab[row_img][:, None] == lab[col_img][None, :]
        m_own_pos = np.where(eq_own, np.float32(0.0), np.float32(-1e30))
        m_own_neg = np.where(eq_own, np.float32(1e30), np.float32(0.0))
        # transposed blocks: anchors = block (c+d)%8, cols = block c images
        mtp_l, mtn_l = [], []
        for d in range(1, NDIAG):
            arow = lab[np.arange(blocks[d] * A, (blocks[d] + 1) * A)]
            eq_t = arow[:, None] == lab[row_img][None, :]
            mtp_l.append(np.where(eq_t, np.float32(0.0), np.float32(-1e30)))
            mtn_l.append(np.where(eq_t, np.float32(1e30), np.float32(0.0)))
        in_maps.append({
            "xa": xa,
            "xr5": xr5,
            "m_own_pos": np.ascontiguousarray(m_own_pos.astype(np.float32)),
            "m_own_neg": np.ascontiguousarray(m_own_neg.astype(np.float32)),
            "m_t_pos": np.ascontiguousarray(
                np.concatenate(mtp_l, axis=1).astype(np.float32)),
            "m_t_neg": np.ascontiguousarray(
                np.concatenate(mtn_l, axis=1).astype(np.float32)),
        })
    res = run_bass_kernel_spmd(
        nc, in_maps, core_ids=list(range(NCORES)), trace=_trace,
        trace_cores=_trace_cores)
    if _trace:
        _CACHE["last_results"] = res

    # host glue: combine per-core min/max partials per anchor block
    an_all = np.full((NCORES, A), np.inf, dtype=np.float32)
    ap_all = np.full((NCORES, A), -np.inf, dtype=np.float32)
    for c in range(NCORES):
        p = res.results[c]["partials"]  # [A, 10]
        for d in range(NDIAG):
            blk = (c + d) % NCORES
            an_all[blk] = np.minimum(an_all[blk], p[:, 2 * d])
            ap_all[blk] = np.maximum(ap_all[blk], p[:, 2 * d + 1])
    loss_vec = np.maximum(
        ap_all.reshape(-1) - an_all.reshape(-1) + np.float32(MARGIN),
        np.float32(0.0))
    return np.asarray(loss_vec.mean(), dtype=np.float32)
